# revision 19
# baseline (speedup 1.0000x reference)
"""Trainium2 Bass kernel for nn_EquivariantLayer (GNN message passing).

out = segment_sum(x[conn_cols] * conn_vals, conn_rows).reshape(N, B*C) @ lw

Feature-major design (8 cores, sharded by output-row range):
  128 partitions = 4 row-quarters x 32 features. Per core:
  - xT [128, V+1]: partition (q, c) holds feature c of all vertices.
  - edges per quarter, degree-sorted ELL per n-sub-shard (host index prep)
  - gpsimd ap_gather (d=1) pulls x values per edge into ELL slots
  - DVE: scale by vals (DMA-replicated), strided slot-reduction -> agg^T
  - gpsimd ap_gather permutes agg rows ELL->proper (n,b) order per sub-shard
  - PE: out^T[64, n] += lw_b^T @ agg^T slices (K=32, row-tiled per quarter)
  host assembles per-core [64, n] outputs into [N, 64].
"""

import sys
from dataclasses import dataclass, field

import numpy as np

sys.path.insert(0, "/opt/trn_rl_repo")

import concourse.bacc as bacc
import concourse.bass as bass
import concourse.mybir as mybir
import concourse.tile as tile
from concourse import library_config
from concourse.bass_utils import run_bass_kernel_spmd

F32 = mybir.dt.float32
I16 = mybir.dt.int16
ADD = mybir.AluOpType.add
MULT = mybir.AluOpType.mult

C_IN, C_OUT, R_IN, R_OUT = 8, 16, 4, 4
P_BINS, T_BINS = 5, 16
B = P_BINS * T_BINS            # 80
C = C_IN * R_IN                # 32 features
KO = C_OUT * R_OUT             # 64
V = 20000
N = 20000
NCORES = 8
P = 128
NQ = 4                         # row quarters (partition groups of 32)


def _angle_rotation(x):
    pif = x.reshape(-1, P_BINS)
    out = np.zeros_like(pif)
    for i in range(R_IN):
        out[i * T_BINS:(i + 1) * T_BINS - 1] = pif[i * T_BINS + 1:(i + 1) * T_BINS]
        out[(i + 1) * T_BINS - 1] = pif[i * T_BINS]
    return out.reshape(-1)


def _kernel_rotation(x):
    pif = x.reshape(-1, T_BINS * P_BINS)
    return np.concatenate([pif[-1:], pif[:-1]], axis=0).reshape(-1)


def _rotation_matrix():
    small = np.zeros((R_IN * B, R_OUT), dtype=np.int64)
    pif = np.arange(R_IN * B, dtype=np.int64)
    small[:, 0] = pif
    for j in range(1, R_OUT):
        pif = _kernel_rotation(_angle_rotation(pif))
        small[:, j] = pif
    return np.concatenate([small + i * R_IN * B for i in range(C_IN)], axis=0)


def _build_lw(weights):
    rot = _rotation_matrix()
    lw = weights[rot.reshape(-1)]
    lw = lw.reshape(C_IN, R_IN, B, R_OUT, C_OUT)
    lw = lw.transpose(2, 0, 1, 4, 3).reshape(B * C_IN * R_IN, C_OUT * R_OUT)
    return np.ascontiguousarray(lw.astype(np.float32))


@dataclass
class Cfg:
    v: int = V
    c: int = C                  # 32 (fixed: 4 quarters x 32 = 128)
    b: int = B
    n_loc: int = N // NCORES    # 2500
    ncores: int = NCORES
    chunk_slots: int = 4096     # gather slots per instruction (per core list)
    sub_n: int = 64             # n per sub-shard
    # filled by prep:
    subs: list = field(default_factory=list)      # (off_n, nn) per sub-shard
    runs: dict = field(default_factory=dict)      # h -> [(d, L)]
    ell_h: list = field(default_factory=list)     # ELL rows per sub-shard
    ts: int = 0                                   # total gather slots per core

    @property
    def quarter_n(self):        # n per quarter (padded)
        qn = (self.n_loc + NQ - 1) // NQ
        return ((qn + self.sub_n - 1) // self.sub_n) * self.sub_n \
            if qn % self.sub_n else qn

    @property
    def quarter_real_n(self):
        return (self.n_loc + NQ - 1) // NQ


def prep_host(cfg: Cfg, conn_rows, conn_cols, conn_vals):
    conn_rows = np.asarray(conn_rows)
    conn_cols = np.asarray(conn_cols)
    conn_vals = np.asarray(conn_vals)
    r_loc = cfg.n_loc * cfg.b
    bounds = np.searchsorted(conn_rows, np.arange(cfg.ncores + 1) * r_loc)
    qn_real = cfg.quarter_real_n          # 625
    qn = cfg.quarter_n                    # 768 (multiple of sub_n)
    subs = []
    off = 0
    while off < qn:
        subs.append((off, min(cfg.sub_n, qn - off)))
        off += cfg.sub_n
    cfg.subs = subs

    # per (core, quarter, sub): degree histogram
    percq = {}
    dmax_h = [1] * len(subs)
    for k in range(cfg.ncores):
        lo, hi = int(bounds[k]), int(bounds[k + 1])
        rows = conn_rows[lo:hi].astype(np.int64) - k * r_loc
        n_of = rows // cfg.b
        b_of = rows % cfg.b
        for q in range(NQ):
            m = (n_of >= q * qn_real) & (n_of < (q + 1) * qn_real)
            nq = n_of[m] - q * qn_real
            rq = nq * cfg.b + b_of[m]          # quarter-local row id
            e_lo = lo + np.flatnonzero(m)      # global edge indices
            order = np.argsort(rq, kind="stable")
            rq = rq[order]
            e_lo = e_lo[order]
            deg = np.bincount(rq, minlength=qn * cfg.b)
            rowptr = np.zeros(qn * cfg.b + 1, np.int64)
            np.cumsum(deg, out=rowptr[1:])
            percq[(k, q)] = (deg, rowptr, e_lo)
            for h, (o_n, nn) in enumerate(subs):
                sl = slice(o_n * cfg.b, (o_n + nn) * cfg.b)
                d_eff = np.maximum(deg[sl], 1)
                dmax_h[h] = max(dmax_h[h], int(d_eff.max()))

    # common run lengths L[h][d] across (core, quarter)
    runs = {}
    for h, (o_n, nn) in enumerate(subs):
        cnt = np.zeros(dmax_h[h] + 1, np.int64)
        for k in range(cfg.ncores):
            for q in range(NQ):
                deg = percq[(k, q)][0]
                d_eff = np.maximum(deg[o_n * cfg.b:(o_n + nn) * cfg.b], 1)
                c = np.bincount(d_eff, minlength=dmax_h[h] + 1)
                cnt = np.maximum(cnt, c)
        runs[h] = [(d, int(-(-cnt[d] // 16) * 16))
                   for d in range(dmax_h[h], 0, -1) if cnt[d] > 0]
    cfg.runs = runs
    cfg.ell_h = [sum(L for _, L in runs[h]) for h in range(len(subs))]
    cfg.ts = sum(sum(d * L for d, L in runs[h]) for h in range(len(subs)))

    core_arrays = []
    for k in range(cfg.ncores):
        # per-core, per-quarter gather cols / vals in ELL slot order +
        # tail permutation indices
        gcols = np.full((NQ, cfg.ts), cfg.v, np.int32)
        gvals = np.zeros((NQ, cfg.ts), np.float32)
        tidx = []
        for q in range(NQ):
            deg, rowptr, e_lo = percq[(k, q)]
            col = 0
            for h, (o_n, nn) in enumerate(subs):
                sl0 = o_n * cfg.b
                sub_rows = nn * cfg.b
                dsub = deg[sl0:sl0 + sub_rows]
                d_eff = np.maximum(dsub, 1)
                ellpos = np.full(sub_rows, -1, np.int64)
                row_off = 0
                for (d, L) in cfg.runs[h]:
                    rws = np.flatnonzero(d_eff == d)
                    ellpos[rws] = row_off + np.arange(len(rws))
                    for kk in range(d):
                        m = dsub[rws] > kk
                        e = e_lo[rowptr[sl0 + rws[m]] + kk]
                        gcols[q, col + kk * L + ellpos[rws[m]] - row_off] = \
                            conn_cols[e]
                        gvals[q, col + kk * L + ellpos[rws[m]] - row_off] = \
                            conn_vals[e]
                    col += d * L
                    row_off += L
                if q == 0 and k == 0:
                    pass
                tidx.append((q, h, ellpos))
        # tail idx arrays per h: [NQ, nn*b] int16 (ell positions)
        tails = []
        for h, (o_n, nn) in enumerate(subs):
            t = np.zeros((NQ, nn * cfg.b), np.int16)
            for (q, hh, ellpos) in tidx:
                if hh == h:
                    t[q] = ellpos.astype(np.int16)
            tails.append(t)
        # wrap into device layouts:
        # gather idx [128, ts/16] i16: partition 32q+16u+j holds elems of
        # quarter-q list with i%16==j (u in {0,1} both replicas of the list)
        gi = np.zeros((P, cfg.ts // 16), np.int16)
        gv = np.zeros((NQ, cfg.ts), np.float32)
        for q in range(NQ):
            w = gcols[q].reshape(cfg.ts // 16, 16).T.astype(np.int16)
            gi[q * 32:q * 32 + 16] = w
            gi[q * 32 + 16:q * 32 + 32] = w
            gv[q] = gvals[q]
        tls = []
        for h, t in enumerate(tails):
            ni = t.shape[1]
            ti = np.zeros((P, ni // 16), np.int16)
            for q in range(NQ):
                w = t[q].reshape(ni // 16, 16).T
                ti[q * 32:q * 32 + 16] = w
                ti[q * 32 + 16:q * 32 + 32] = w
            tls.append(ti)
        core_arrays.append({
            "gath": gi,
            "vals": gv,                     # [4, ts] compact, replicated by DMA
            "tails": tls,
        })
    return core_arrays


def build(cfg: Cfg):
    nc = bacc.Bacc("TRN2", debug=False, num_devices=cfg.ncores)

    xt_d = nc.dram_tensor("xT", [P, cfg.v + 1], F32, kind="ExternalInput")
    lw_d = nc.dram_tensor("lwd", [P, cfg.b * KO], F32, kind="ExternalInput")
    gi_d = nc.dram_tensor("gath", [P, cfg.ts // 16], I16, kind="ExternalInput")
    gv_d = nc.dram_tensor("vals", [NQ, cfg.ts], F32, kind="ExternalInput")
    tl_d = [nc.dram_tensor(f"tail{h}", [P, (nn * cfg.b) // 16], I16,
                           kind="ExternalInput")
            for h, (_, nn) in enumerate(cfg.subs)]
    out_d = nc.dram_tensor("outT", [KO, NQ * cfg.quarter_n], F32,
                           kind="ExternalOutput")

    with tile.TileContext(nc) as tc:
        with tc.tile_pool(name="const", bufs=1) as const, \
             tc.tile_pool(name="msgp", bufs=2) as msgp, \
             tc.tile_pool(name="valp", bufs=2) as valp, \
             tc.tile_pool(name="idxp", bufs=2) as idxp, \
             tc.tile_pool(name="otp", bufs=2) as otp, \
             tc.tile_pool(name="agg", bufs=1) as aggp, \
             tc.tile_pool(name="aggP", bufs=1) as aggPp, \
             tc.tile_pool(name="ps", bufs=4, space="PSUM") as psp:
            nc.gpsimd.load_library(library_config.ap_gather)
            xt_sb = const.tile([P, cfg.v + 1], F32)
            nc.sync.dma_start(out=xt_sb[:], in_=xt_d[:])
            lw_sb = const.tile([P, cfg.b * KO], F32)
            nc.sync.dma_start(out=lw_sb[:], in_=lw_d[:])

            col = 0          # slot offset (per core list)
            for h, (o_n, nn) in enumerate(cfg.subs):
                ell_h = cfg.ell_h[h]
                agg = aggp.tile([P, ell_h], F32, tag="agg")
                row_off = 0
                for (d, L) in cfg.runs[h]:
                    rps = max(4, (cfg.chunk_slots // d) // 4 * 4)
                    r0 = 0
                    while r0 < L:
                        rs = min(rps, L - r0)
                        if rs % 4:
                            rs += 4 - rs % 4
                            rs = min(rs, L - r0)
                        nidx = d * rs
                        # idx slice: within-run chunking needs slot-major
                        # per-chunk layout -> host laid run as [d, L]; a
                        # chunk [d, r0:r0+rs] is strided. Use whole-run
                        # instructions when possible; else per-slot calls.
                        assert r0 == 0 and rs == L, \
                            "run must fit one chunk; raise chunk_slots"
                        gi_sb = idxp.tile([P, nidx // 16], I16, tag="gidx")
                        nc.sync.dma_start(
                            out=gi_sb[:],
                            in_=gi_d[:, (col // 16):(col + nidx) // 16])
                        idx_ap = gi_sb[:]
                        vals_sb = valp.tile([P, nidx], F32, tag="vals")
                        # replicate quarter vals across its 32 partitions
                        for q in range(NQ):
                            nc.sync.dma_start(
                                out=vals_sb[q * 32:(q + 1) * 32, :],
                                in_=gv_d[q:q + 1, col:col + nidx]
                                    .to_broadcast([32, nidx]))
                        msg = msgp.tile([P, nidx], F32, tag="msg")
                        nc.gpsimd.ap_gather(
                            msg[:].rearrange("p (i d) -> p i d", d=1),
                            xt_sb[:].rearrange("p (i d) -> p i d", d=1),
                            idx_ap, channels=P, num_elems=cfg.v + 1,
                            d=1, num_idxs=nidx)
                        nc.vector.tensor_tensor(
                            out=msg[:], in0=msg[:], in1=vals_sb[:], op=MULT)
                        dst = agg[:, row_off:row_off + L]
                        if d == 1:
                            nc.vector.tensor_copy(out=dst, in_=msg[:])
                        else:
                            nc.vector.tensor_tensor(
                                out=dst, in0=msg[:, 0:L], in1=msg[:, L:2 * L],
                                op=ADD)
                            for kk in range(2, d):
                                nc.vector.tensor_tensor(
                                    out=dst, in0=dst,
                                    in1=msg[:, kk * L:(kk + 1) * L], op=ADD)
                        col += nidx
                        r0 += rs
                    row_off += L

                # tail: permute ELL -> proper (n-major, b) order
                ni = nn * cfg.b
                ti_sb = idxp.tile([P, ni // 16], I16, tag="tidx")
                nc.sync.dma_start(out=ti_sb[:], in_=tl_d[h][:])
                aggP = aggPp.tile([P, ni], F32, tag="aggP")
                nc.gpsimd.ap_gather(
                    aggP[:].rearrange("p (i d) -> p i d", d=1),
                    agg[:].rearrange("p (i d) -> p i d", d=1),
                    ti_sb[:], channels=P, num_elems=ell_h, d=1, num_idxs=ni)

                # matmuls: per quarter q: out^T[64, nn] += lw_b^T @ agg slice
                for q in range(NQ):
                    ps = psp.tile([KO, nn], F32, tag="ps", space="PSUM")
                    rhs_q = aggP[q * 32:(q + 1) * 32, :]
                    for bb in range(cfg.b):
                        rhs = rhs_q.rearrange(
                            "p (n b) -> p n b", b=cfg.b)[:, :, bb]
                        nc.tensor.matmul(
                            out=ps[:],
                            lhsT=lw_sb[q * 32:(q + 1) * 32,
                                       bb * KO:(bb + 1) * KO],
                            rhs=rhs,
                            start=(bb == 0), stop=(bb == cfg.b - 1),
                            tile_position=(q * 32, 0))
                    ot = otp.tile([KO, nn], F32, tag="otile")
                    nc.vector.tensor_copy(out=ot[:], in_=ps[:])
                    nc.sync.dma_start(
                        out=out_d[:, (q * cfg.quarter_n + o_n):
                                  (q * cfg.quarter_n + o_n + nn)],
                        in_=ot[:])

    nc.compile()
    return nc


_CACHE = {}


def run(x, conn_rows, conn_cols, conn_vals, weights, trace=False):
    cfg = Cfg()
    core_arrays = prep_host(cfg, conn_rows, conn_cols, conn_vals)

    x = np.asarray(x, np.float32)
    xt = np.zeros((P, cfg.v + 1), np.float32)
    for p in range(P):
        xt[p, :cfg.v] = x[:, p % cfg.c]
    lw = _build_lw(np.asarray(weights, np.float32))
    lw_dev = np.zeros((P, cfg.b * KO), np.float32)
    for p in range(P):
        c = p % cfg.c
        for bb in range(cfg.b):
            lw_dev[p, bb * KO:(bb + 1) * KO] = lw[bb * cfg.c + c]

    key = (cfg.ts, tuple(cfg.ell_h),
           tuple(tuple(r) for h in range(len(cfg.subs))
                 for r in cfg.runs[h]))
    if key not in _CACHE:
        _CACHE[key] = build(cfg)
    nc = _CACHE[key]

    in_maps = []
    for k in range(cfg.ncores):
        m = {"xT": xt, "lwd": lw_dev,
             "gath": core_arrays[k]["gath"],
             "vals": core_arrays[k]["vals"]}
        for h in range(len(cfg.subs)):
            m[f"tail{h}"] = core_arrays[k]["tails"][h]
        in_maps.append(m)
    res = run_bass_kernel_spmd(nc, in_maps, list(range(cfg.ncores)),
                               trace=trace)

    out = np.empty((N, KO), np.float32)
    qn = cfg.quarter_n
    qr = cfg.quarter_real_n
    for k in range(cfg.ncores):
        ot = res.results[k]["outT"]     # [64, NQ*qn]
        for q in range(NQ):
            base = k * cfg.n_loc + q * qr
            cnt = min(qr, cfg.n_loc - q * qr)
            if cnt > 0:
                out[base:base + cnt] = ot[:, q * qn:q * qn + cnt].T
    return out, res


def kernel(x, conn_rows, conn_cols, conn_vals, weights):
    out, _ = run(x, conn_rows, conn_cols, conn_vals, weights)
    return out
